# revision 1
# baseline (speedup 1.0000x reference)
"""Masked multi-head self-attention kernel for 8 Trainium2 NeuronCores.

Full module: qkv projection -> causal softmax attention (16 heads) -> out
projection, for x[4, 2048, 1024].

Sharding: core c handles batch b = c//2 and heads h0 = (c%2)*8 .. h0+8.
QKV projection + attention are fully local to a core.  The out projection
contracts over all 16 heads' channels, so the two cores of a batch exchange
their attention outputs with pairwise AllGathers (chunked over heads and
query blocks for overlap) and each computes half of the output columns.
Each core returns out[b][:, half].T (transposed: [512, 2048]); the host
reassembles.  Inputs are re-laid-out per core on the host: x transposed,
qkv weight columns / out-proj rows sliced and permuted to the gather order.
"""

import math
import os
import sys

for _p in ("/opt/trn_rl_repo", "/root/.axon_site/_ro/trn_rl_repo"):
    if os.path.isdir(_p) and _p not in sys.path:
        sys.path.insert(0, _p)
        break

import ml_dtypes
import numpy as np

import concourse.bass as bass
import concourse.mybir as mybir
import concourse.tile as tile
from concourse import bacc
from concourse.bass_utils import run_bass_kernel_spmd

B, T, C, H = 4, 2048, 1024, 16
D = 64                 # head dim
NCORES = 8
HPC = H // 2           # heads per core = 8
CPC = HPC * D          # channels per core = 512
P = 128                # partitions
QB = 512               # query block
NQB = T // QB          # 4
KC = C // P            # contraction chunks for C = 8
NTT = T // P           # 16 t-tiles
SCALE = 1.0 / math.sqrt(D)

F32 = mybir.dt.float32
F32R = mybir.dt.float32r
BF16 = mybir.dt.bfloat16
EXP = mybir.ActivationFunctionType.Exp

_CACHE = {}


def build():
    nc = bacc.Bacc("TRN2", num_devices=NCORES, debug=False)

    xT = nc.dram_tensor("xT", [C, T], BF16, kind="ExternalInput")
    wqkv = nc.dram_tensor("wqkv", [C, 3 * CPC], BF16, kind="ExternalInput")
    bqkv = nc.dram_tensor("bqkv", [1, 3 * CPC], F32, kind="ExternalInput")
    wout = nc.dram_tensor("wout", [C, CPC], BF16, kind="ExternalInput")
    bout = nc.dram_tensor("bout", [1, CPC], F32, kind="ExternalInput")
    outT = nc.dram_tensor("outT", [CPC, T], F32, kind="ExternalOutput")

    groups = [[0, 1], [2, 3], [4, 5], [6, 7]]

    with tile.TileContext(nc) as tc:
        with (
            tc.tile_pool(name="const", bufs=1) as constp,
            tc.tile_pool(name="ytp", bufs=1) as ytp,
            tc.tile_pool(name="vaugp", bufs=1) as vaugp,
            tc.tile_pool(name="dram", bufs=1, space="DRAM") as dramp,
        ):
            # per-partition bias layouts: bq_sb[p, n] = bqkv[n*128 + p]
            bq_sb = constp.tile([P, 12], F32, tag="bq")
            nc.sync.dma_start(
                bq_sb[:].rearrange("p (o n) -> p o n", o=1),
                bqkv.ap().rearrange("o (n p) -> p o n", p=P),
            )
            bo_sb = constp.tile([P, 4], F32, tag="bo")
            nc.sync.dma_start(
                bo_sb[:].rearrange("p (o n) -> p o n", o=1),
                bout.ap().rearrange("o (n p) -> p o n", p=P),
            )
            ones_f32 = constp.tile([P, P], F32, tag="ones")
            nc.vector.memset(ones_f32[:], 1.0)

            # Q^T,K^T: 8 chunks of [128 ch, 2048 t] (Q: 0-3, K: 4-7)
            yts = [
                ytp.tile([P, T], BF16, name=f"yt{n}", tag=f"yt{n}")
                for n in range(8)
            ]
            # V natural (+ones col) per head: ktile k at cols k*65
            vaugs = [
                vaugp.tile([P, NTT * 65], BF16, name=f"vaug{h}", tag=f"vaug{h}")
                for h in range(HPC)
            ]
            vaug3s = [
                v[:].rearrange("p (k c) -> p k c", c=65) for v in vaugs
            ]
            for h in range(HPC):
                nc.vector.tensor_copy(
                    vaug3s[h][:, :, 64:65],
                    ones_f32[:, 0:NTT].rearrange("p (a b) -> p a b", b=1),
                )

            # ---------------- stage 1: qkv projection, V ----------------
            with (
                tc.tile_pool(name="xtp", bufs=1) as xtp,
                tc.tile_pool(name="wtile", bufs=10) as wtp,
                tc.tile_pool(name="wvp", bufs=1) as wvp,
                tc.tile_pool(name="ps_y", bufs=4, space="PSUM") as psy,
                tc.tile_pool(name="ps_v", bufs=4, space="PSUM") as psv,
            ):
                # x^T chunks resident in SBUF: [128 ch, 2048 t] each
                xts = [
                    xtp.tile([P, T], BF16, name=f"xt{cc}", tag=f"xt{cc}")
                    for cc in range(KC)
                ]
                for cc in range(KC):
                    nc.sync.dma_start(
                        xts[cc][:], xT[cc * P:(cc + 1) * P, :]
                    )

                wv_tiles = []
                for kc in range(KC):
                    wv = wvp.tile(
                        [P, CPC], BF16, name=f"wv{kc}", tag=f"wv{kc}"
                    )
                    nc.sync.dma_start(
                        wv[:], wqkv[kc * P:(kc + 1) * P, 2 * CPC:3 * CPC]
                    )
                    wv_tiles.append(wv)

                def qk_chunk(n):
                    # kc outer so each weight tile serves 4 matmuls
                    pys = [
                        psy.tile([P, QB], F32, name=f"py{n}_{i}", tag="py")
                        for i in range(4)
                    ]
                    for kc in range(KC):
                        wt = wtp.tile([P, P], BF16, tag="wt")
                        nc.sync.dma_start(
                            wt[:],
                            wqkv[kc * P:(kc + 1) * P, n * P:(n + 1) * P],
                        )
                        for tc4 in range(4):
                            nc.tensor.matmul(
                                pys[tc4][:],
                                wt[:],
                                xts[kc][:, tc4 * QB:(tc4 + 1) * QB],
                                start=(kc == 0),
                                stop=(kc == KC - 1),
                            )
                    for tc4 in range(4):
                        nc.vector.tensor_scalar_add(
                            yts[n][:, tc4 * QB:(tc4 + 1) * QB],
                            pys[tc4][:],
                            bq_sb[:, n:n + 1],
                        )

                def v_block(tts):
                    # V natural: out[t, vch] with x^T tiles stationary;
                    # V bias is folded into the output bias on the host
                    for tt in tts:
                        pv = psv.tile([P, CPC], F32, tag="pv")
                        for kc in range(KC):
                            nc.tensor.matmul(
                                pv[:],
                                xts[kc][:, tt * P:(tt + 1) * P],
                                wv_tiles[kc][:],
                                start=(kc == 0),
                                stop=(kc == KC - 1),
                            )
                        for h in range(HPC):
                            nc.vector.tensor_copy(
                                vaug3s[h][:, tt, 0:64],
                                pv[:, h * 64:h * 64 + 64],
                            )

                for blk in range(4):
                    qk_chunk(blk)
                    qk_chunk(4 + blk)
                    v_block(range(4 * blk, 4 * blk + 4))

            # ---------------- stage 2+3: attention, gather, out proj ----
            with (
                tc.tile_pool(name="pt", bufs=36) as ptp,
                tc.tile_pool(name="recip", bufs=4) as recipp,
                tc.tile_pool(name="bc", bufs=3) as bcp,
                tc.tile_pool(name="atv", bufs=3) as atvp,
                tc.tile_pool(name="w2", bufs=1) as w2p,
                tc.tile_pool(name="agr", bufs=3) as agrp,
                tc.tile_pool(name="outsb", bufs=3) as outsbp,
                tc.tile_pool(name="ps_s", bufs=4, space="PSUM") as pss,
                tc.tile_pool(name="ps_a", bufs=2, space="PSUM") as psa,
                tc.tile_pool(name="ps_o", bufs=2, space="PSUM") as pso,
            ):
                w2sb = w2p.tile([P, KC * CPC], BF16, tag="w2")
                nc.sync.dma_start(
                    w2sb[:].rearrange("p (c n) -> p c n", n=CPC),
                    wout.ap().rearrange("(c p) n -> p c n", p=P),
                )
                w23 = w2sb[:].rearrange("p (c n) -> p c n", n=CPC)

                def s_pass(qb, h):
                    """score matmuls + exp (+causal mask) for one head/qblock.
                    Diagonal k-tiles first so their exp+mask (on the PV
                    critical path) complete while off-diagonal scores stream.
                    """
                    qt = yts[h // 2]
                    kt_c = yts[4 + h // 2]
                    poff = (h % 2) * 64
                    nkt = 4 * qb + 4
                    kts = list(range(4 * qb, nkt)) + list(range(0, 4 * qb))
                    pts = []
                    for kt in kts:
                        j = kt - 4 * qb  # >=0 on diagonal tiles
                        qoff = max(j, 0) * P
                        ps = pss.tile([P, QB], F32, tag="ps")
                        nc.tensor.matmul(
                            ps[:, qoff:QB],
                            kt_c[poff:poff + 64, kt * P:(kt + 1) * P],
                            qt[poff:poff + 64, qb * QB + qoff:(qb + 1) * QB],
                            start=True, stop=True,
                        )
                        pt = ptp.tile([P, QB], BF16, tag="pt")
                        nc.scalar.activation(
                            pt[:, qoff:QB], ps[:, qoff:QB], EXP, scale=SCALE
                        )
                        if j >= 0:
                            # zero where q < k (also fills the stale prefix)
                            nc.gpsimd.affine_select(
                                out=pt[:],
                                in_=pt[:],
                                compare_op=mybir.AluOpType.is_ge,
                                fill=0.0,
                                base=-j * P,
                                pattern=[[1, QB]],
                                channel_multiplier=-1,
                            )
                        pts.append((kt, pt))
                    return pts

                def pv_pass(qb, h, pts, ag_in, row):
                    pa = psa.tile([P, QB], F32, tag="pa")
                    for i, (kt, pt) in enumerate(pts):
                        nc.tensor.matmul(
                            pa[0:65, :],
                            vaug3s[h][:, kt, :],
                            pt[:],
                            start=(i == 0),
                            stop=(i == len(pts) - 1),
                        )
                    sums = recipp.tile([1, QB], F32, tag="sums")
                    nc.vector.tensor_copy(sums[:], pa[64:65, :])
                    recip = recipp.tile([1, QB], F32, tag="recip")
                    nc.vector.reciprocal_approx_fast(recip[:], sums[:])
                    bc = bcp.tile([64, QB], F32, tag="bc")
                    nc.gpsimd.partition_broadcast(bc[:], recip[:])
                    atv = atvp.tile([64, QB], BF16, tag="atv")
                    nc.vector.tensor_mul(atv[:], pa[0:64, :], bc[:])
                    nc.sync.dma_start(
                        ag_in[row * 64:(row + 1) * 64, :], atv[:]
                    )

                def gather(ag_in, ag_out):
                    nc.gpsimd.collective_compute(
                        "AllGather",
                        mybir.AluOpType.bypass,
                        replica_groups=groups,
                        ins=[ag_in.opt()],
                        outs=[ag_out.opt()],
                    )

                def out_proj(qb, ag_outs):
                    # w_out rows are host-permuted to match the gathered
                    # row order [even0-3, odd0-3, even4-5, odd4-5, ...]
                    agr3s = []
                    for gi, ago in enumerate(ag_outs):
                        ncch = 2 * (GGRP[gi][1] - GGRP[gi][0]) * 64 // P
                        agr = agrp.tile(
                            [P, ncch * QB], BF16,
                            name=f"agr{qb}_{gi}", tag=f"agr{gi}",
                        )
                        nc.sync.dma_start(
                            agr[:].rearrange("p (c n) -> p c n", n=QB),
                            ago[:].rearrange("(c p) n -> p c n", p=P),
                        )
                        agr3s.append(
                            agr[:].rearrange("p (c n) -> p c n", n=QB)
                        )
                    # chunk cc -> (gather buffer, sub-chunk)
                    ccmap = [(0, 0), (0, 1), (0, 2), (0, 3),
                             (1, 0), (1, 1), (2, 0), (2, 1)]
                    for oc in range(4):
                        po = pso.tile([P, QB], F32, tag="po")
                        for cc in range(KC):
                            gi, sub = ccmap[cc]
                            nc.tensor.matmul(
                                po[:],
                                w23[:, cc, oc * P:(oc + 1) * P],
                                agr3s[gi][:, sub, :],
                                start=(cc == 0),
                                stop=(cc == KC - 1),
                            )
                        osb = outsbp.tile([P, QB], F32, tag="osb")
                        nc.vector.tensor_scalar_add(
                            osb[:], po[:], bo_sb[:, oc:oc + 1]
                        )
                        nc.sync.dma_start(
                            outT[oc * P:(oc + 1) * P, qb * QB:(qb + 1) * QB],
                            osb[:],
                        )

                # gather groups: heads 0-3, heads 4-5, heads 6-7
                GGRP = [(0, 4), (4, 6), (6, 8)]

                pending_outproj = None
                for qb in range(NQB):
                    ag_ins = [
                        dramp.tile(
                            [(e - s) * 64, QB], BF16,
                            name=f"agin{qb}_{i}", tag=f"agin{qb}_{i}",
                        )
                        for i, (s, e) in enumerate(GGRP)
                    ]
                    ag_outs = [
                        dramp.tile(
                            [2 * (e - s) * 64, QB], BF16,
                            name=f"agout{qb}_{i}", tag=f"agout{qb}_{i}",
                        )
                        for i, (s, e) in enumerate(GGRP)
                    ]
                    grp_of = {}
                    for i, (s, e) in enumerate(GGRP):
                        for h in range(s, e):
                            grp_of[h] = (i, h - s)
                    prev = None
                    for h in range(HPC):
                        cur = s_pass(qb, h)
                        if h == 3 and pending_outproj is not None:
                            # previous qblock's out-projection: its gather
                            # waits hide behind this qblock's score matmuls
                            pending_outproj()
                            pending_outproj = None
                        if prev is not None:
                            hp = h - 1
                            gi, row = grp_of[hp]
                            pv_pass(qb, hp, prev, ag_ins[gi], row)
                            if hp in (3, 5):
                                gather(ag_ins[gi], ag_outs[gi])
                        prev = cur
                    gi, row = grp_of[HPC - 1]
                    pv_pass(qb, HPC - 1, prev, ag_ins[gi], row)
                    gather(ag_ins[gi], ag_outs[gi])
                    pending_outproj = (
                        lambda qb=qb, ag_outs=ag_outs: out_proj(qb, ag_outs)
                    )
                pending_outproj()

    nc.compile()
    return nc


def kernel(x, w_qkv, b_qkv, w_out, b_out):
    x = np.asarray(x, dtype=np.float32)
    w_qkv = np.asarray(w_qkv, dtype=np.float32)
    b_qkv = np.asarray(b_qkv, dtype=np.float32)
    w_out = np.asarray(w_out, dtype=np.float32)
    b_out = np.asarray(b_out, dtype=np.float32)

    if "nc" not in _CACHE:
        _CACHE["nc"] = build()
    nc = _CACHE["nc"]

    # V bias passes through softmax unchanged; fold it into the out bias
    bv_perm_all = b_qkv[2 * C:3 * C]

    in_maps = []
    for c in range(NCORES):
        b = c // 2
        h0 = (c % 2) * HPC
        cols = slice(h0 * D, h0 * D + CPC)
        wq = np.concatenate(
            [w_qkv[:, cols], w_qkv[:, C:][:, cols], w_qkv[:, 2 * C:][:, cols]],
            axis=1,
        )
        bq = np.concatenate(
            [b_qkv[cols], b_qkv[C:][cols], b_qkv[2 * C:][cols]]
        ).reshape(1, 3 * CPC)
        half = slice((c % 2) * CPC, (c % 2) * CPC + CPC)
        wo = w_out[:, half]
        # rows permuted to the gathered channel order:
        # [even h0-3, odd h0-3, even h4-5, odd h4-5, even h6-7, odd h6-7]
        wo_perm = np.concatenate(
            [wo[0:256], wo[512:768],
             wo[256:384], wo[768:896],
             wo[384:512], wo[896:1024]], axis=0
        )
        bout_eff = b_out[half] + bv_perm_all @ w_out[:, half]
        in_maps.append({
            "xT": np.ascontiguousarray(x[b].T.astype(ml_dtypes.bfloat16)),
            "wqkv": np.ascontiguousarray(wq.astype(ml_dtypes.bfloat16)),
            "bqkv": np.ascontiguousarray(bq),
            "wout": np.ascontiguousarray(wo_perm.astype(ml_dtypes.bfloat16)),
            "bout": np.ascontiguousarray(bout_eff).reshape(1, CPC),
        })

    kwargs = {}
    tdir = os.environ.get("KERNEL_TRACE_DIR")
    if tdir:
        kwargs = dict(trace=True, tmpdir=tdir)
    res = run_bass_kernel_spmd(
        nc, in_maps, core_ids=list(range(NCORES)), **kwargs
    )
    _CACHE["last_results"] = res

    out = np.empty((B, T, C), dtype=np.float32)
    for c in range(NCORES):
        b = c // 2
        half = slice((c % 2) * CPC, (c % 2) * CPC + CPC)
        out[b][:, half] = res.results[c]["outT"].T
    return out



# revision 13
# speedup vs baseline: 1.1238x; 1.1238x over previous
"""Masked multi-head self-attention kernel for 8 Trainium2 NeuronCores.

Full module: qkv projection -> causal softmax attention (16 heads) -> out
projection, for x[4, 2048, 1024].

Sharding: core c handles batch b = c//2 and heads h0 = (c%2)*8 .. h0+8.

Structure (vs the serial baseline):
- Scores for an even/odd head pair run as two concurrent matmuls on
  disjoint PE row groups (K=64, partitions 0-63 / 64-127) into one 2-bank
  PSUM tile; a single Exp activation covers both heads, causally trimmed.
- PV runs as two concurrent matmuls on disjoint PE column groups (M=64).
  Softmax denominators come from M=1 ones-matmuls at column strips 0/32.
- The causal mask only touches the 128-column diagonal block.
- Normalization (recip of denominator, broadcast via a K=33 masked
  matmul, multiply) is deferred into the next pair's stream so its
  latency never head-of-line-blocks the PE queue.
- The QKV projection is emitted as fine-grained (2-matmul) filler chunks
  interleaved with attention so the tensor engine stays dense; t-block 3
  K/V projections are deferred into the last query block, which is
  otherwise activation-bound.
- The final out-projection is split so only its last 4 contraction
  chunks sit behind the final AllGather.
"""

import math
import os
import sys

for _p in ("/opt/trn_rl_repo", "/root/.axon_site/_ro/trn_rl_repo"):
    if os.path.isdir(_p) and _p not in sys.path:
        sys.path.insert(0, _p)
        break

import ml_dtypes
import numpy as np

import concourse.bass as bass
import concourse.mybir as mybir
import concourse.tile as tile
from concourse import bacc
from concourse.bass_utils import run_bass_kernel_spmd

B, T, C, H = 4, 2048, 1024, 16
D = 64                 # head dim
NCORES = 8
HPC = H // 2           # heads per core = 8
NPAIR = HPC // 2       # head pairs per core = 4
CPC = HPC * D          # channels per core = 512
P = 128                # partitions
QB = 512               # query block
NQB = T // QB          # 4
KC = C // P            # contraction chunks for C = 8
NTT = T // P           # 16 t-tiles
SCALE = 1.0 / math.sqrt(D)

F32 = mybir.dt.float32
BF16 = mybir.dt.bfloat16
EXP = mybir.ActivationFunctionType.Exp

_CACHE = {}
DEFER = os.environ.get("KERNEL_DEFER", "1") == "1"
NORM_K33 = os.environ.get("KERNEL_NORM", "k33") == "k33"
UNITS_GEN = os.environ.get("KERNEL_UNITS", "coarse") == "gen"
QB3_DEFER = os.environ.get("KERNEL_QB3", "defer") == "defer"
SPLITOP = os.environ.get("KERNEL_SPLITOP", "1") == "1"


def build():
    nc = bacc.Bacc("TRN2", num_devices=NCORES, debug=False)

    xT = nc.dram_tensor("xT", [C, T], BF16, kind="ExternalInput")
    wqkv = nc.dram_tensor("wqkv", [C, 3 * CPC], BF16, kind="ExternalInput")
    bqkv = nc.dram_tensor("bqkv", [1, 3 * CPC], F32, kind="ExternalInput")
    wout = nc.dram_tensor("wout", [C, CPC], BF16, kind="ExternalInput")
    bout = nc.dram_tensor("bout", [1, CPC], F32, kind="ExternalInput")
    mh2 = nc.dram_tensor("mh2", [2, P], BF16, kind="ExternalInput")
    outT = nc.dram_tensor("outT", [CPC, T], F32, kind="ExternalOutput")

    groups = [[0, 1], [2, 3], [4, 5], [6, 7]]

    with tile.TileContext(nc) as tc:
        with (
            tc.tile_pool(name="const", bufs=1) as constp,
            tc.tile_pool(name="stat", bufs=1) as statp,
            tc.tile_pool(name="ptp", bufs=6) as ptp,
            tc.tile_pool(name="bcp", bufs=2) as bcp,
            tc.tile_pool(name="atvp", bufs=2) as atvp,
            tc.tile_pool(name="agrp", bufs=2) as agrp,
            tc.tile_pool(name="outsbp", bufs=2) as outsbp,
            tc.tile_pool(name="dram", bufs=1, space="DRAM") as dramp,
            tc.tile_pool(name="pssp", bufs=2, space="PSUM") as pssp,
            tc.tile_pool(name="pap", bufs=2, space="PSUM") as pap,
            tc.tile_pool(name="dnp", bufs=1, space="PSUM") as dnp,
            tc.tile_pool(name="flp", bufs=1, space="PSUM") as flp,
        ):
            # ---------------- persistent SBUF + input DMAs ----------------
            xts = [
                statp.tile([P, T], BF16, name=f"xt{i}", tag=f"xt{i}")
                for i in range(KC)
            ]
            wqk_sb = [
                statp.tile([P, 2 * CPC], BF16, name=f"wqk{i}", tag=f"wqk{i}")
                for i in range(KC)
            ]
            wv_sb = [
                statp.tile([P, CPC], BF16, name=f"wv{i}", tag=f"wv{i}")
                for i in range(KC)
            ]
            for kc in range(KC):
                nc.sync.dma_start(
                    wqk_sb[kc][:], wqkv[kc * P:(kc + 1) * P, 0:2 * CPC]
                )
                nc.sync.dma_start(
                    xts[kc][:, 0:QB], xT[kc * P:(kc + 1) * P, 0:QB]
                )
                nc.sync.dma_start(
                    wv_sb[kc][:], wqkv[kc * P:(kc + 1) * P, 2 * CPC:3 * CPC]
                )
            for tb in range(1, NQB):
                for kc in range(KC):
                    nc.sync.dma_start(
                        xts[kc][:, tb * QB:(tb + 1) * QB],
                        xT[kc * P:(kc + 1) * P, tb * QB:(tb + 1) * QB],
                    )

            # constants (after the stage-1-critical DMAs)
            bq_sb = constp.tile([P, 12], F32, tag="bq")
            nc.sync.dma_start(
                bq_sb[:].rearrange("p (o n) -> p o n", o=1),
                bqkv.ap().rearrange("o (n p) -> p o n", p=P),
            )
            bo_sb = constp.tile([P, 4], F32, tag="bo")
            nc.sync.dma_start(
                bo_sb[:].rearrange("p (o n) -> p o n", o=1),
                bout.ap().rearrange("o (n p) -> p o n", p=P),
            )
            ones_col = constp.tile([P, 1], BF16, tag="ones")
            nc.vector.memset(ones_col[:], 1.0)
            # broadcast masks at rows 0 and 32 (K=33 matmul), rest zero
            mh = constp.tile([P, P], BF16, tag="mh")
            nc.vector.memset(mh[:], 0.0)
            nc.sync.dma_start(mh[0:1, :], mh2[0:1, :])
            nc.sync.dma_start(mh[32:33, :], mh2[1:2, :])
            nc.sync.dma_start(mh[1:2, :], mh2[1:2, :])

            # Q^T chunks (pair p: n=p) and K^T chunks (n=4+p), [128 ch, T]
            yts = [
                statp.tile([P, T], BF16, name=f"yt{n}", tag=f"yt{n}")
                for n in range(8)
            ]
            # V natural per t-tile: [128 kpos, 512 vch]
            vtt = [
                statp.tile([P, CPC], BF16, name=f"vtt{t}", tag=f"vtt{t}")
                for t in range(NTT)
            ]

            w2sb = statp.tile([P, KC * CPC], BF16, tag="w2")
            nc.sync.dma_start(
                w2sb[:].rearrange("p (c n) -> p c n", n=CPC),
                wout.ap().rearrange("(c p) n -> p c n", p=P),
            )
            w23 = w2sb[:].rearrange("p (c n) -> p c n", n=CPC)

            # dsb pool slots must have rows 1-31 zero forever (the K=33
            # broadcast matmul streams rows 0-32); zero them once.
            for z in range(2):
                dz = bcp.tile([P, QB], BF16, name=f"dsbz{z}", tag="dsb")
                nc.vector.memset(dz[:], 0.0)

            # ---------------- stage-1 units (as 2-matmul generators) ------
            def qk_unit_gen(n, tb):
                py = flp.tile([P, QB], F32, name=f"py{n}_{tb}", tag="fl")
                for kc0 in range(0, KC, 2):
                    for kc in (kc0, kc0 + 1):
                        nc.tensor.matmul(
                            py[:],
                            wqk_sb[kc][:, n * P:(n + 1) * P],
                            xts[kc][:, tb * QB:(tb + 1) * QB],
                            start=(kc == 0),
                            stop=(kc == KC - 1),
                        )
                    yield
                nc.vector.tensor_scalar_add(
                    yts[n][:, tb * QB:(tb + 1) * QB], py[:], bq_sb[:, n:n + 1]
                )
                yield

            def v_unit_gen(tt):
                pv = flp.tile([P, CPC], F32, name=f"pv{tt}", tag="fl")
                for kc0 in range(0, KC, 2):
                    for kc in (kc0, kc0 + 1):
                        nc.tensor.matmul(
                            pv[:],
                            xts[kc][:, tt * P:(tt + 1) * P],
                            wv_sb[kc][:],
                            start=(kc == 0),
                            stop=(kc == KC - 1),
                        )
                    yield
                nc.vector.tensor_copy(vtt[tt][:], pv[:])
                yield

            class UnitRunner:
                """Drains stage-1 unit generators, one 2-matmul chunk per
                step() call, units strictly in order (they share psum)."""

                def __init__(self):
                    self.queue = []
                    self.cur = None

                def add(self, gens):
                    self.queue.extend(gens)

                def step(self, nchunks=1):
                    if not UNITS_GEN:
                        # coarse mode: run one full unit per call
                        if self.queue:
                            for _ in self.queue.pop(0)():
                                pass
                        return
                    for _ in range(nchunks):
                        if self.cur is None:
                            if not self.queue:
                                return
                            self.cur = self.queue.pop(0)()
                        try:
                            next(self.cur)
                        except StopIteration:
                            self.cur = None

                def drain(self):
                    while self.cur is not None or self.queue:
                        self.step()

            runner = UnitRunner()

            def tb_gens(tb, only=None):
                gens = []
                for n in range(4):
                    if only in (None, "k"):
                        gens.append(lambda n=n, tb=tb: qk_unit_gen(4 + n, tb))
                    if only in (None, "q"):
                        gens.append(lambda n=n, tb=tb: qk_unit_gen(n, tb))
                if only in (None, "v"):
                    for tt in range(4 * tb, 4 * tb + 4):
                        gens.append(lambda tt=tt: v_unit_gen(tt))
                return gens

            # upfront: t-block 0 dense
            runner.add(tb_gens(0))
            runner.drain()

            # ---------------- attention ----------------
            GGRP = [(0, 2), (2, 3), (3, 4)]  # pair ranges per gather group
            pending_outproj = None
            pending_norm = None
            oacc = None

            def out_proj(qb, ag_outs, phase=None, acc=None):
                """Standard: full 8-chunk contraction.  split phase 'A':
                chunks 0-3 (group 0) into SBUF acc; phase 'B': chunks 4-7
                plus acc."""
                agr3s = {}
                ccmap = [(0, 0), (0, 1), (0, 2), (0, 3),
                         (1, 0), (1, 1), (2, 0), (2, 1)]
                ccs = list(range(KC))
                if phase == "A":
                    ccs = list(range(0, 4))
                elif phase == "B":
                    ccs = list(range(4, KC))
                gneeded = sorted({ccmap[cc][0] for cc in ccs})
                for gi in gneeded:
                    s, e = GGRP[gi]
                    ncch = 2 * (e - s)
                    agr = agrp.tile(
                        [P, ncch * QB], BF16,
                        name=f"agr{qb}_{gi}{phase or ''}", tag=f"agr{gi}",
                    )
                    nc.sync.dma_start(
                        agr[:].rearrange("p (c n) -> p c n", n=QB),
                        ag_outs[gi][:].rearrange("(c p) n -> p c n", p=P),
                    )
                    agr3s[gi] = agr[:].rearrange("p (c n) -> p c n", n=QB)
                for oc in range(4):
                    po = flp.tile(
                        [P, QB], F32, name=f"po{qb}_{oc}{phase or ''}",
                        tag="fl",
                    )
                    for j, cc in enumerate(ccs):
                        gi, sub = ccmap[cc]
                        nc.tensor.matmul(
                            po[:],
                            w23[:, cc, oc * P:(oc + 1) * P],
                            agr3s[gi][:, sub, :],
                            start=(j == 0),
                            stop=(j == len(ccs) - 1),
                        )
                    if phase == "A":
                        nc.vector.tensor_copy(acc[oc][:], po[:])
                    else:
                        osb = outsbp.tile([P, QB], F32, tag="osb")
                        nc.vector.tensor_scalar_add(
                            osb[:], po[:], bo_sb[:, oc:oc + 1]
                        )
                        if phase == "B":
                            nc.vector.tensor_add(osb[:], osb[:], acc[oc][:])
                        nc.sync.dma_start(
                            outT[oc * P:(oc + 1) * P,
                                 qb * QB:(qb + 1) * QB],
                            osb[:],
                        )

            last_ag_outs = None
            for qb in range(NQB):
                nkt = 4 * qb + 4
                if qb == 0:
                    kts = [0, 1, 2, 3]
                else:
                    kts = list(range(4 * qb)) + [4 * qb + j for j in range(4)]
                qoffs = [
                    P * (kt - 4 * qb) if kt >= 4 * qb else 0 for kt in kts
                ]
                diag = [kt >= 4 * qb for kt in kts]

                ag_ins = [
                    dramp.tile(
                        [(e - s) * P, QB], BF16,
                        name=f"agin{qb}_{i}", tag=f"agin{qb}_{i}",
                    )
                    for i, (s, e) in enumerate(GGRP)
                ]
                ag_outs = [
                    dramp.tile(
                        [2 * (e - s) * P, QB], BF16,
                        name=f"agout{qb}_{i}", tag=f"agout{qb}_{i}",
                    )
                    for i, (s, e) in enumerate(GGRP)
                ]
                last_ag_outs = ag_outs

                # per-(qb, pair) stage-1 generators
                if qb == 0:
                    allg = tb_gens(1)
                    per_pair = [allg[3 * p:3 * p + 3] for p in range(4)]
                elif qb == 1:
                    allg = tb_gens(2)
                    per_pair = [allg[3 * p:3 * p + 3] for p in range(4)]
                elif qb == 2:
                    if QB3_DEFER:
                        qg = tb_gens(3, only="q")
                        per_pair = [[qg[p]] for p in range(4)]
                    else:
                        allg = tb_gens(3)
                        per_pair = [allg[3 * p:3 * p + 3] for p in range(4)]
                elif QB3_DEFER:
                    kg = tb_gens(3, only="k")
                    vg = tb_gens(3, only="v")
                    per_pair = [[kg[0]] + vg, [kg[1]], [kg[2]], [kg[3]]]
                else:
                    per_pair = [[] for _ in range(4)]

                if qb == NQB - 1:
                    oacc = [
                        statp.tile([P, QB], F32, name=f"oacc{oc}",
                                   tag=f"oacc{oc}")
                        for oc in range(4)
                    ]

                def gather(gi, ag_ins=ag_ins, ag_outs=ag_outs):
                    nc.gpsimd.collective_compute(
                        "AllGather",
                        mybir.AluOpType.bypass,
                        replica_groups=groups,
                        ins=[ag_ins[gi].opt()],
                        outs=[ag_outs[gi].opt()],
                    )

                # chunks per kt-step so each qb's generator queue drains
                # in time: qb0 16 steps/60 chunks, qb1 32/60, qb2 48/20,
                # qb3 64/40 (front-loaded for pair 0's diagonal needs).
                rate = {0: 4, 1: 2, 2: 1, 3: 1}[qb]

                def attn_pair(p, qb=qb, nkt=nkt, kts=kts, qoffs=qoffs,
                              diag=diag, ag_ins=ag_ins, ag_outs=ag_outs,
                              per_pair=per_pair, gather=gather, rate=rate):
                    nonlocal pending_norm
                    he, ho = 2 * p, 2 * p + 1
                    qt = yts[p]
                    ktc = yts[4 + p]
                    runner.add(per_pair[p])
                    pa = pap.tile([P, QB], F32, name=f"pa{qb}_{p}", tag="pa")
                    dn = dnp.tile([P, QB], F32, name=f"dn{qb}_{p}", tag="dn")
                    PIPE = 3
                    pend = []

                    def flush_one():
                        kt, qoff, pt3, i = pend.pop(0)
                        first = i == 0
                        last = i == nkt - 1
                        nc.tensor.matmul(
                            pa[0:64, qoff:QB],
                            vtt[kt][:, he * D:he * D + D],
                            pt3[:, 0, qoff:QB],
                            start=first, stop=last,
                            skip_group_check=True,
                        )
                        nc.tensor.matmul(
                            pa[64:128, qoff:QB],
                            vtt[kt][:, ho * D:ho * D + D],
                            pt3[:, 1, qoff:QB],
                            start=first, stop=last,
                            skip_group_check=True,
                        )
                        for par, row in ((0, 0), (1, 32)):
                            nc.tensor.matmul(
                                dn[row:row + 1, qoff:QB],
                                ones_col[:],
                                pt3[:, par, qoff:QB],
                                start=first, stop=last,
                                skip_group_check=True,
                                tile_position=(0, row),
                            )

                    for i, kt in enumerate(kts):
                        qoff = qoffs[i]
                        ps = pssp.tile(
                            [P, 2 * QB], F32, name=f"ps{qb}_{p}_{i}", tag="ps"
                        )
                        ps3 = ps.rearrange("p (c n) -> p c n", c=2)
                        nc.tensor.matmul(
                            ps3[:, 0, qoff:QB],
                            ktc[0:64, kt * P:(kt + 1) * P],
                            qt[0:64, qb * QB + qoff:(qb + 1) * QB],
                            start=True, stop=True,
                        )
                        nc.tensor.matmul(
                            ps3[:, 1, qoff:QB],
                            ktc[64:128, kt * P:(kt + 1) * P],
                            qt[64:128, qb * QB + qoff:(qb + 1) * QB],
                            start=True, stop=True,
                        )
                        pt = ptp.tile(
                            [P, 2 * QB], BF16, name=f"pt{qb}_{p}_{i}", tag="pt"
                        )
                        pt3 = pt.rearrange("p (c n) -> p c n", c=2)
                        nc.scalar.activation(
                            pt3[:, :, qoff:QB], ps3[:, :, qoff:QB], EXP,
                            scale=SCALE,
                        )
                        if diag[i]:
                            nc.gpsimd.affine_select(
                                out=pt3[:, :, qoff:qoff + P],
                                in_=pt3[:, :, qoff:qoff + P],
                                compare_op=mybir.AluOpType.is_ge,
                                fill=0.0,
                                base=0,
                                pattern=[[0, 2], [1, P]],
                                channel_multiplier=-1,
                            )
                        pend.append((kt, qoff, pt3, i))
                        if len(pend) > PIPE:
                            flush_one()
                        if i == 1 and pending_norm is not None:
                            pending_norm()
                            pending_norm = None
                        runner.step(3 if qb == 3 and p == 0 else rate)

                    while pend:
                        flush_one()

                    def norm(qb=qb, p=p, pa=pa, dn=dn, ag_ins=ag_ins,
                             ag_outs=ag_outs, gather=gather):
                        # den rows 0/32 -> SBUF; K=33 masked matmul
                        # broadcasts them to partition halves in dn; copy
                        # out; one base-0 recip; multiply; ship.
                        dsb = bcp.tile([P, QB], BF16, name=f"dsb{qb}_{p}",
                                       tag="dsb")
                        nc.vector.tensor_copy(dsb[0:1, :], dn[0:1, :])
                        nc.vector.tensor_copy(dsb[32:33, :], dn[32:33, :])
                        if NORM_K33:
                            nc.tensor.matmul(dn[:, :], mh[0:33, :],
                                             dsb[0:33, :],
                                             start=True, stop=True,
                                             skip_group_check=True)
                        else:
                            nc.sync.dma_start(dsb[1:2, :], dsb[32:33, :])
                            nc.tensor.matmul(dn[:, :], mh[0:2, :],
                                             dsb[0:2, :],
                                             start=True, stop=True,
                                             skip_group_check=True)
                        bcd = bcp.tile([P, QB], F32, name=f"bcd{qb}_{p}",
                                       tag="bcd")
                        nc.vector.tensor_copy(bcd[:], dn[:, :])
                        bc2 = bcp.tile([P, QB], F32, name=f"bc2{qb}_{p}",
                                       tag="bc2")
                        nc.vector.reciprocal_approx_fast(bc2[:], bcd[:])
                        atv = atvp.tile([P, QB], BF16, name=f"atv{qb}_{p}",
                                        tag="atv")
                        nc.vector.tensor_mul(atv[:], pa[:], bc2[:])
                        gi = 0 if p < 2 else (1 if p == 2 else 2)
                        row = p if p < 2 else 0
                        nc.sync.dma_start(
                            ag_ins[gi][row * P:(row + 1) * P, :], atv[:]
                        )
                        if p != 0:
                            gather(gi)
                        if SPLITOP and qb == NQB - 1 and p == 1:
                            # tail-shortening: first half of the final
                            # out-projection once group 0 is gathered
                            out_proj(qb, ag_outs, phase="A", acc=oacc)

                    if DEFER:
                        pending_norm = norm
                    else:
                        norm()

                for p in range(NPAIR):
                    attn_pair(p)
                    if p == 1 and pending_outproj is not None:
                        pending_outproj()
                        pending_outproj = None

                if qb < NQB - 1:
                    pending_outproj = (
                        lambda qb=qb, ag_outs=ag_outs: out_proj(qb, ag_outs)
                    )

            # drain the tail: last pair's norm + gather, then phase B
            if pending_norm is not None:
                pending_norm()
                pending_norm = None
            runner.drain()
            if SPLITOP:
                out_proj(NQB - 1, last_ag_outs, phase="B", acc=oacc)
            else:
                out_proj(NQB - 1, last_ag_outs)

    nc.compile()
    return nc


def kernel(x, w_qkv, b_qkv, w_out, b_out):
    x = np.asarray(x, dtype=np.float32)
    w_qkv = np.asarray(w_qkv, dtype=np.float32)
    b_qkv = np.asarray(b_qkv, dtype=np.float32)
    w_out = np.asarray(w_out, dtype=np.float32)
    b_out = np.asarray(b_out, dtype=np.float32)

    if "nc" not in _CACHE:
        _CACHE["nc"] = build()
    nc = _CACHE["nc"]

    # V bias passes through softmax unchanged; fold it into the out bias
    bv_all = b_qkv[2 * C:3 * C]

    in_maps = []
    for c in range(NCORES):
        b = c // 2
        h0 = (c % 2) * HPC
        cols = slice(h0 * D, h0 * D + CPC)
        wq = np.concatenate(
            [w_qkv[:, cols], w_qkv[:, C:][:, cols], w_qkv[:, 2 * C:][:, cols]],
            axis=1,
        )
        bq = np.concatenate(
            [b_qkv[cols], b_qkv[C:][cols], b_qkv[2 * C:][cols]]
        ).reshape(1, 3 * CPC)
        half = slice((c % 2) * CPC, (c % 2) * CPC + CPC)
        wo = w_out[:, half]
        # rows permuted to the gathered channel order:
        # [even h0-3, odd h0-3, even h4-5, odd h4-5, even h6-7, odd h6-7]
        wo_perm = np.concatenate(
            [wo[0:256], wo[512:768],
             wo[256:384], wo[768:896],
             wo[384:512], wo[896:1024]], axis=0
        )
        bout_eff = b_out[half] + bv_all @ w_out[:, half]
        mh2v = np.concatenate([
            np.concatenate([np.ones((1, 64)), np.zeros((1, 64))], 1),
            np.concatenate([np.zeros((1, 64)), np.ones((1, 64))], 1),
        ]).astype(ml_dtypes.bfloat16)
        in_maps.append({
            "mh2": np.ascontiguousarray(mh2v),
            "xT": np.ascontiguousarray(x[b].T.astype(ml_dtypes.bfloat16)),
            "wqkv": np.ascontiguousarray(wq.astype(ml_dtypes.bfloat16)),
            "bqkv": np.ascontiguousarray(bq),
            "wout": np.ascontiguousarray(wo_perm.astype(ml_dtypes.bfloat16)),
            "bout": np.ascontiguousarray(bout_eff).reshape(1, CPC),
        })

    kwargs = {}
    tdir = os.environ.get("KERNEL_TRACE_DIR")
    if tdir:
        kwargs = dict(trace=True, tmpdir=tdir)
    res = run_bass_kernel_spmd(
        nc, in_maps, core_ids=list(range(NCORES)), **kwargs
    )
    _CACHE["last_results"] = res

    out = np.empty((B, T, C), dtype=np.float32)
    for c in range(NCORES):
        b = c // 2
        half = slice((c % 2) * CPC, (c % 2) * CPC + CPC)
        out[b][:, half] = res.results[c]["outT"].T
    return out


# revision 15
# speedup vs baseline: 1.1591x; 1.0313x over previous
"""Masked multi-head self-attention kernel for 8 Trainium2 NeuronCores.

Full module: qkv projection -> causal softmax attention (16 heads) -> out
projection, for x[4, 2048, 1024].

Sharding: core c handles batch b = c//2 and heads h0 = (c%2)*8 .. h0+8.

Structure (vs the serial baseline):
- Scores for an even/odd head pair run as two concurrent matmuls on
  disjoint PE row groups (K=64, partitions 0-63 / 64-127) into one 2-bank
  PSUM tile; a single Exp activation covers both heads, causally trimmed.
- PV runs as two concurrent matmuls on disjoint PE column groups (M=64).
  Softmax denominators come from M=1 ones-matmuls at column strips 0/32.
- The causal mask only touches the 128-column diagonal block.
- Normalization (recip of denominator, broadcast via a K=33 masked
  matmul, multiply) is deferred into the next pair's stream so its
  latency never head-of-line-blocks the PE queue.
- The QKV projection is emitted as fine-grained (2-matmul) filler chunks
  interleaved with attention so the tensor engine stays dense; t-block 3
  K/V projections are deferred into the last query block, which is
  otherwise activation-bound.
- The final out-projection is split so only its last 4 contraction
  chunks sit behind the final AllGather.
"""

import math
import os
import sys

for _p in ("/opt/trn_rl_repo", "/root/.axon_site/_ro/trn_rl_repo"):
    if os.path.isdir(_p) and _p not in sys.path:
        sys.path.insert(0, _p)
        break

import ml_dtypes
import numpy as np

import concourse.bass as bass
import concourse.mybir as mybir
import concourse.tile as tile
from concourse import bacc
from concourse.bass_utils import run_bass_kernel_spmd

B, T, C, H = 4, 2048, 1024, 16
D = 64                 # head dim
NCORES = 8
HPC = H // 2           # heads per core = 8
NPAIR = HPC // 2       # head pairs per core = 4
CPC = HPC * D          # channels per core = 512
P = 128                # partitions
QB = 512               # query block
NQB = T // QB          # 4
KC = C // P            # contraction chunks for C = 8
NTT = T // P           # 16 t-tiles
SCALE = 1.0 / math.sqrt(D)

F32 = mybir.dt.float32
BF16 = mybir.dt.bfloat16
EXP = mybir.ActivationFunctionType.Exp

_CACHE = {}
DEFER = os.environ.get("KERNEL_DEFER", "1") == "1"
NORM_K33 = os.environ.get("KERNEL_NORM", "k33") == "k33"
UNITS_GEN = os.environ.get("KERNEL_UNITS", "coarse") == "gen"
QB3_DEFER = os.environ.get("KERNEL_QB3", "defer") == "defer"
SPLITOP = os.environ.get("KERNEL_SPLITOP", "1") == "1"


def build():
    nc = bacc.Bacc("TRN2", num_devices=NCORES, debug=False)

    xT = nc.dram_tensor("xT", [C, T], BF16, kind="ExternalInput")
    wqkv = nc.dram_tensor("wqkv", [C, 3 * CPC], BF16, kind="ExternalInput")
    bqkv = nc.dram_tensor("bqkv", [1, 3 * CPC], F32, kind="ExternalInput")
    wout = nc.dram_tensor("wout", [C, CPC], BF16, kind="ExternalInput")
    bout = nc.dram_tensor("bout", [1, CPC], F32, kind="ExternalInput")
    mh2 = nc.dram_tensor("mh2", [2, P], BF16, kind="ExternalInput")
    outT = nc.dram_tensor("outT", [CPC, T], F32, kind="ExternalOutput")

    groups = [[0, 1], [2, 3], [4, 5], [6, 7]]

    with tile.TileContext(nc) as tc:
        with (
            tc.tile_pool(name="const", bufs=1) as constp,
            tc.tile_pool(name="stat", bufs=1) as statp,
            tc.tile_pool(name="ptp", bufs=6) as ptp,
            tc.tile_pool(name="bcp", bufs=2) as bcp,
            tc.tile_pool(name="atvp", bufs=2) as atvp,
            tc.tile_pool(name="agrp", bufs=2) as agrp,
            tc.tile_pool(name="outsbp", bufs=2) as outsbp,
            tc.tile_pool(name="dram", bufs=1, space="DRAM") as dramp,
            tc.tile_pool(name="pssp", bufs=2, space="PSUM") as pssp,
            tc.tile_pool(name="pap", bufs=2, space="PSUM") as pap,
            tc.tile_pool(name="dnp", bufs=1, space="PSUM") as dnp,
            tc.tile_pool(name="flp", bufs=1, space="PSUM") as flp,
        ):
            # ---------------- persistent SBUF + input DMAs ----------------
            xts = [
                statp.tile([P, T], BF16, name=f"xt{i}", tag=f"xt{i}")
                for i in range(KC)
            ]
            wqk_sb = [
                statp.tile([P, 2 * CPC], BF16, name=f"wqk{i}", tag=f"wqk{i}")
                for i in range(KC)
            ]
            wv_sb = [
                statp.tile([P, CPC], BF16, name=f"wv{i}", tag=f"wv{i}")
                for i in range(KC)
            ]
            QS = [nc.sync, nc.scalar, nc.gpsimd]
            qi = [0]

            def dmaq(*args):
                QS[qi[0] % 3].dma_start(*args)
                qi[0] += 1

            for kc in range(KC):
                dmaq(wqk_sb[kc][:], wqkv[kc * P:(kc + 1) * P, 0:2 * CPC])
                dmaq(xts[kc][:, 0:QB], xT[kc * P:(kc + 1) * P, 0:QB])
            for kc in range(KC):
                dmaq(wv_sb[kc][:], wqkv[kc * P:(kc + 1) * P, 2 * CPC:3 * CPC])
            for tb in range(1, NQB):
                for kc in range(KC):
                    dmaq(
                        xts[kc][:, tb * QB:(tb + 1) * QB],
                        xT[kc * P:(kc + 1) * P, tb * QB:(tb + 1) * QB],
                    )

            # constants (after the stage-1-critical DMAs)
            bq_sb = constp.tile([P, 12], F32, tag="bq")
            nc.sync.dma_start(
                bq_sb[:].rearrange("p (o n) -> p o n", o=1),
                bqkv.ap().rearrange("o (n p) -> p o n", p=P),
            )
            bo_sb = constp.tile([P, 4], F32, tag="bo")
            nc.sync.dma_start(
                bo_sb[:].rearrange("p (o n) -> p o n", o=1),
                bout.ap().rearrange("o (n p) -> p o n", p=P),
            )
            ones_col = constp.tile([P, 1], BF16, tag="ones")
            nc.vector.memset(ones_col[:], 1.0)
            # broadcast masks at rows 0 and 32 (K=33 matmul), rest zero
            mh = constp.tile([P, P], BF16, tag="mh")
            nc.vector.memset(mh[:], 0.0)
            nc.sync.dma_start(mh[0:1, :], mh2[0:1, :])
            nc.sync.dma_start(mh[32:33, :], mh2[1:2, :])
            nc.sync.dma_start(mh[1:2, :], mh2[1:2, :])

            # Q^T chunks (pair p: n=p) and K^T chunks (n=4+p), [128 ch, T]
            yts = [
                statp.tile([P, T], BF16, name=f"yt{n}", tag=f"yt{n}")
                for n in range(8)
            ]
            # V natural per t-tile: [128 kpos, 512 vch]
            vtt = [
                statp.tile([P, CPC], BF16, name=f"vtt{t}", tag=f"vtt{t}")
                for t in range(NTT)
            ]

            w2sb = statp.tile([P, KC * CPC], BF16, tag="w2")
            nc.sync.dma_start(
                w2sb[:].rearrange("p (c n) -> p c n", n=CPC),
                wout.ap().rearrange("(c p) n -> p c n", p=P),
            )
            w23 = w2sb[:].rearrange("p (c n) -> p c n", n=CPC)

            # dsb pool slots must have rows 1-31 zero forever (the K=33
            # broadcast matmul streams rows 0-32); zero them once.
            for z in range(2):
                dz = bcp.tile([P, QB], BF16, name=f"dsbz{z}", tag="dsb")
                nc.vector.memset(dz[:], 0.0)

            # ---------------- stage-1 units (as 2-matmul generators) ------
            def qk_unit_gen(n, tb):
                py = flp.tile([P, QB], F32, name=f"py{n}_{tb}", tag="fl")
                for kc0 in range(0, KC, 2):
                    for kc in (kc0, kc0 + 1):
                        nc.tensor.matmul(
                            py[:],
                            wqk_sb[kc][:, n * P:(n + 1) * P],
                            xts[kc][:, tb * QB:(tb + 1) * QB],
                            start=(kc == 0),
                            stop=(kc == KC - 1),
                        )
                    yield
                nc.vector.tensor_scalar_add(
                    yts[n][:, tb * QB:(tb + 1) * QB], py[:], bq_sb[:, n:n + 1]
                )
                yield

            def v_unit_gen(tt):
                pv = flp.tile([P, CPC], F32, name=f"pv{tt}", tag="fl")
                for kc0 in range(0, KC, 2):
                    for kc in (kc0, kc0 + 1):
                        nc.tensor.matmul(
                            pv[:],
                            xts[kc][:, tt * P:(tt + 1) * P],
                            wv_sb[kc][:],
                            start=(kc == 0),
                            stop=(kc == KC - 1),
                        )
                    yield
                nc.vector.tensor_copy(vtt[tt][:], pv[:])
                yield

            class UnitRunner:
                """Drains stage-1 unit generators, one 2-matmul chunk per
                step() call, units strictly in order (they share psum)."""

                def __init__(self):
                    self.queue = []
                    self.cur = None

                def add(self, gens):
                    self.queue.extend(gens)

                def step(self, nchunks=1):
                    if not UNITS_GEN:
                        # coarse mode: run one full unit per call
                        if self.queue:
                            for _ in self.queue.pop(0)():
                                pass
                        return
                    for _ in range(nchunks):
                        if self.cur is None:
                            if not self.queue:
                                return
                            self.cur = self.queue.pop(0)()
                        try:
                            next(self.cur)
                        except StopIteration:
                            self.cur = None

                def drain(self):
                    while self.cur is not None or self.queue:
                        self.step()

            runner = UnitRunner()

            def tb_gens(tb, only=None):
                gens = []
                for n in range(4):
                    if only in (None, "k"):
                        gens.append(lambda n=n, tb=tb: qk_unit_gen(4 + n, tb))
                    if only in (None, "q"):
                        gens.append(lambda n=n, tb=tb: qk_unit_gen(n, tb))
                if only in (None, "v"):
                    for tt in range(4 * tb, 4 * tb + 4):
                        gens.append(lambda tt=tt: v_unit_gen(tt))
                return gens

            # upfront: t-block 0 dense
            runner.add(tb_gens(0))
            runner.drain()

            # ---------------- attention ----------------
            GGRP = [(0, 2), (2, 3), (3, 4)]  # pair ranges per gather group
            pending_outproj = None
            pending_norm = None
            oacc = None

            def out_proj(qb, ag_outs, phase=None, acc=None):
                """Standard: full 8-chunk contraction.  split phase 'A':
                chunks 0-3 (group 0) into SBUF acc; phase 'B': chunks 4-7
                plus acc."""
                agr3s = {}
                ccmap = [(0, 0), (0, 1), (0, 2), (0, 3),
                         (1, 0), (1, 1), (2, 0), (2, 1)]
                ccs = list(range(KC))
                if phase == "A":
                    ccs = list(range(0, 6))
                elif phase == "B":
                    ccs = list(range(6, KC))
                gneeded = sorted({ccmap[cc][0] for cc in ccs})
                for gi in gneeded:
                    s, e = GGRP[gi]
                    ncch = 2 * (e - s)
                    agr = agrp.tile(
                        [P, ncch * QB], BF16,
                        name=f"agr{qb}_{gi}{phase or ''}", tag=f"agr{gi}",
                    )
                    nc.sync.dma_start(
                        agr[:].rearrange("p (c n) -> p c n", n=QB),
                        ag_outs[gi][:].rearrange("(c p) n -> p c n", p=P),
                    )
                    agr3s[gi] = agr[:].rearrange("p (c n) -> p c n", n=QB)
                for oc in range(4):
                    po = flp.tile(
                        [P, QB], F32, name=f"po{qb}_{oc}{phase or ''}",
                        tag="fl",
                    )
                    for j, cc in enumerate(ccs):
                        gi, sub = ccmap[cc]
                        nc.tensor.matmul(
                            po[:],
                            w23[:, cc, oc * P:(oc + 1) * P],
                            agr3s[gi][:, sub, :],
                            start=(j == 0),
                            stop=(j == len(ccs) - 1),
                        )
                    if phase == "A":
                        nc.vector.tensor_copy(acc[oc][:], po[:])
                    else:
                        osb = outsbp.tile([P, QB], F32, tag="osb")
                        nc.vector.tensor_scalar_add(
                            osb[:], po[:], bo_sb[:, oc:oc + 1]
                        )
                        if phase == "B":
                            nc.vector.tensor_add(osb[:], osb[:], acc[oc][:])
                        nc.sync.dma_start(
                            outT[oc * P:(oc + 1) * P,
                                 qb * QB:(qb + 1) * QB],
                            osb[:],
                        )

            last_ag_outs = None
            for qb in range(NQB):
                nkt = 4 * qb + 4
                if qb == 0:
                    kts = [0, 1, 2, 3]
                else:
                    kts = list(range(4 * qb)) + [4 * qb + j for j in range(4)]
                qoffs = [
                    P * (kt - 4 * qb) if kt >= 4 * qb else 0 for kt in kts
                ]
                diag = [kt >= 4 * qb for kt in kts]

                ag_ins = [
                    dramp.tile(
                        [(e - s) * P, QB], BF16,
                        name=f"agin{qb}_{i}", tag=f"agin{qb}_{i}",
                    )
                    for i, (s, e) in enumerate(GGRP)
                ]
                ag_outs = [
                    dramp.tile(
                        [2 * (e - s) * P, QB], BF16,
                        name=f"agout{qb}_{i}", tag=f"agout{qb}_{i}",
                    )
                    for i, (s, e) in enumerate(GGRP)
                ]
                last_ag_outs = ag_outs

                # per-(qb, pair) stage-1 generators
                if qb == 0:
                    allg = tb_gens(1)
                    per_pair = [allg[3 * p:3 * p + 3] for p in range(4)]
                elif qb == 1:
                    allg = tb_gens(2)
                    per_pair = [allg[3 * p:3 * p + 3] for p in range(4)]
                elif qb == 2:
                    if QB3_DEFER:
                        qg = tb_gens(3, only="q")
                        per_pair = [[qg[p]] for p in range(4)]
                    else:
                        allg = tb_gens(3)
                        per_pair = [allg[3 * p:3 * p + 3] for p in range(4)]
                elif QB3_DEFER:
                    kg = tb_gens(3, only="k")
                    vg = tb_gens(3, only="v")
                    per_pair = [[kg[0]] + vg, [kg[1]], [kg[2]], [kg[3]]]
                else:
                    per_pair = [[] for _ in range(4)]

                if qb == NQB - 1:
                    oacc = [
                        statp.tile([P, QB], F32, name=f"oacc{oc}",
                                   tag=f"oacc{oc}")
                        for oc in range(4)
                    ]

                def gather(gi, ag_ins=ag_ins, ag_outs=ag_outs):
                    nc.gpsimd.collective_compute(
                        "AllGather",
                        mybir.AluOpType.bypass,
                        replica_groups=groups,
                        ins=[ag_ins[gi].opt()],
                        outs=[ag_outs[gi].opt()],
                    )

                # chunks per kt-step so each qb's generator queue drains
                # in time: qb0 16 steps/60 chunks, qb1 32/60, qb2 48/20,
                # qb3 64/40 (front-loaded for pair 0's diagonal needs).
                rate = {0: 4, 1: 2, 2: 1, 3: 1}[qb]

                def attn_pair(p, qb=qb, nkt=nkt, kts=kts, qoffs=qoffs,
                              diag=diag, ag_ins=ag_ins, ag_outs=ag_outs,
                              per_pair=per_pair, gather=gather, rate=rate):
                    nonlocal pending_norm
                    he, ho = 2 * p, 2 * p + 1
                    qt = yts[p]
                    ktc = yts[4 + p]
                    runner.add(per_pair[p])
                    pa = pap.tile([P, QB], F32, name=f"pa{qb}_{p}", tag="pa")
                    dn = dnp.tile([P, QB], F32, name=f"dn{qb}_{p}", tag="dn")
                    PIPE = 3
                    pend = []

                    def flush_one():
                        kt, qoff, pt3, i = pend.pop(0)
                        first = i == 0
                        last = i == nkt - 1
                        nc.tensor.matmul(
                            pa[0:64, qoff:QB],
                            vtt[kt][:, he * D:he * D + D],
                            pt3[:, 0, qoff:QB],
                            start=first, stop=last,
                            skip_group_check=True,
                        )
                        nc.tensor.matmul(
                            pa[64:128, qoff:QB],
                            vtt[kt][:, ho * D:ho * D + D],
                            pt3[:, 1, qoff:QB],
                            start=first, stop=last,
                            skip_group_check=True,
                        )
                        for par, row in ((0, 0), (1, 32)):
                            nc.tensor.matmul(
                                dn[row:row + 1, qoff:QB],
                                ones_col[:],
                                pt3[:, par, qoff:QB],
                                start=first, stop=last,
                                skip_group_check=True,
                                tile_position=(0, row),
                            )

                    for i, kt in enumerate(kts):
                        qoff = qoffs[i]
                        ps = pssp.tile(
                            [P, 2 * QB], F32, name=f"ps{qb}_{p}_{i}", tag="ps"
                        )
                        ps3 = ps.rearrange("p (c n) -> p c n", c=2)
                        nc.tensor.matmul(
                            ps3[:, 0, qoff:QB],
                            ktc[0:64, kt * P:(kt + 1) * P],
                            qt[0:64, qb * QB + qoff:(qb + 1) * QB],
                            start=True, stop=True,
                        )
                        nc.tensor.matmul(
                            ps3[:, 1, qoff:QB],
                            ktc[64:128, kt * P:(kt + 1) * P],
                            qt[64:128, qb * QB + qoff:(qb + 1) * QB],
                            start=True, stop=True,
                        )
                        pt = ptp.tile(
                            [P, 2 * QB], BF16, name=f"pt{qb}_{p}_{i}", tag="pt"
                        )
                        pt3 = pt.rearrange("p (c n) -> p c n", c=2)
                        nc.scalar.activation(
                            pt3[:, :, qoff:QB], ps3[:, :, qoff:QB], EXP,
                            scale=SCALE,
                        )
                        if diag[i]:
                            nc.gpsimd.affine_select(
                                out=pt3[:, :, qoff:qoff + P],
                                in_=pt3[:, :, qoff:qoff + P],
                                compare_op=mybir.AluOpType.is_ge,
                                fill=0.0,
                                base=0,
                                pattern=[[0, 2], [1, P]],
                                channel_multiplier=-1,
                            )
                        pend.append((kt, qoff, pt3, i))
                        if len(pend) > PIPE:
                            flush_one()
                        if i == 1 and pending_norm is not None:
                            pending_norm()
                            pending_norm = None
                        runner.step(3 if qb == 3 and p == 0 else rate)

                    while pend:
                        flush_one()

                    def norm(qb=qb, p=p, pa=pa, dn=dn, ag_ins=ag_ins,
                             ag_outs=ag_outs, gather=gather):
                        # den rows 0/32 -> SBUF; K=33 masked matmul
                        # broadcasts them to partition halves in dn; copy
                        # out; one base-0 recip; multiply; ship.
                        dsb = bcp.tile([P, QB], BF16, name=f"dsb{qb}_{p}",
                                       tag="dsb")
                        nc.vector.tensor_copy(dsb[0:1, :], dn[0:1, :])
                        nc.vector.tensor_copy(dsb[32:33, :], dn[32:33, :])
                        if NORM_K33:
                            nc.tensor.matmul(dn[:, :], mh[0:33, :],
                                             dsb[0:33, :],
                                             start=True, stop=True,
                                             skip_group_check=True)
                        else:
                            nc.sync.dma_start(dsb[1:2, :], dsb[32:33, :])
                            nc.tensor.matmul(dn[:, :], mh[0:2, :],
                                             dsb[0:2, :],
                                             start=True, stop=True,
                                             skip_group_check=True)
                        bcd = bcp.tile([P, QB], F32, name=f"bcd{qb}_{p}",
                                       tag="bcd")
                        nc.vector.tensor_copy(bcd[:], dn[:, :])
                        bc2 = bcp.tile([P, QB], F32, name=f"bc2{qb}_{p}",
                                       tag="bc2")
                        nc.vector.reciprocal_approx_fast(bc2[:], bcd[:])
                        atv = atvp.tile([P, QB], BF16, name=f"atv{qb}_{p}",
                                        tag="atv")
                        nc.vector.tensor_mul(atv[:], pa[:], bc2[:])
                        gi = 0 if p < 2 else (1 if p == 2 else 2)
                        row = p if p < 2 else 0
                        nc.sync.dma_start(
                            ag_ins[gi][row * P:(row + 1) * P, :], atv[:]
                        )
                        if p != 0:
                            gather(gi)
                        if SPLITOP and qb == NQB - 1 and p == 2:
                            # tail-shortening: most of the final
                            # out-projection once groups 0+1 are gathered
                            out_proj(qb, ag_outs, phase="A", acc=oacc)

                    if DEFER:
                        pending_norm = norm
                    else:
                        norm()

                for p in range(NPAIR):
                    attn_pair(p)
                    if p == 1 and pending_outproj is not None:
                        pending_outproj()
                        pending_outproj = None

                if qb < NQB - 1:
                    pending_outproj = (
                        lambda qb=qb, ag_outs=ag_outs: out_proj(qb, ag_outs)
                    )

            # drain the tail: last pair's norm + gather, then phase B
            if pending_norm is not None:
                pending_norm()
                pending_norm = None
            runner.drain()
            if SPLITOP:
                out_proj(NQB - 1, last_ag_outs, phase="B", acc=oacc)
            else:
                out_proj(NQB - 1, last_ag_outs)

    nc.compile()
    return nc


def kernel(x, w_qkv, b_qkv, w_out, b_out):
    x = np.asarray(x, dtype=np.float32)
    w_qkv = np.asarray(w_qkv, dtype=np.float32)
    b_qkv = np.asarray(b_qkv, dtype=np.float32)
    w_out = np.asarray(w_out, dtype=np.float32)
    b_out = np.asarray(b_out, dtype=np.float32)

    if "nc" not in _CACHE:
        _CACHE["nc"] = build()
    nc = _CACHE["nc"]

    # V bias passes through softmax unchanged; fold it into the out bias
    bv_all = b_qkv[2 * C:3 * C]

    in_maps = []
    for c in range(NCORES):
        b = c // 2
        h0 = (c % 2) * HPC
        cols = slice(h0 * D, h0 * D + CPC)
        wq = np.concatenate(
            [w_qkv[:, cols], w_qkv[:, C:][:, cols], w_qkv[:, 2 * C:][:, cols]],
            axis=1,
        )
        bq = np.concatenate(
            [b_qkv[cols], b_qkv[C:][cols], b_qkv[2 * C:][cols]]
        ).reshape(1, 3 * CPC)
        half = slice((c % 2) * CPC, (c % 2) * CPC + CPC)
        wo = w_out[:, half]
        # rows permuted to the gathered channel order:
        # [even h0-3, odd h0-3, even h4-5, odd h4-5, even h6-7, odd h6-7]
        wo_perm = np.concatenate(
            [wo[0:256], wo[512:768],
             wo[256:384], wo[768:896],
             wo[384:512], wo[896:1024]], axis=0
        )
        bout_eff = b_out[half] + bv_all @ w_out[:, half]
        mh2v = np.concatenate([
            np.concatenate([np.ones((1, 64)), np.zeros((1, 64))], 1),
            np.concatenate([np.zeros((1, 64)), np.ones((1, 64))], 1),
        ]).astype(ml_dtypes.bfloat16)
        in_maps.append({
            "mh2": np.ascontiguousarray(mh2v),
            "xT": np.ascontiguousarray(x[b].T.astype(ml_dtypes.bfloat16)),
            "wqkv": np.ascontiguousarray(wq.astype(ml_dtypes.bfloat16)),
            "bqkv": np.ascontiguousarray(bq),
            "wout": np.ascontiguousarray(wo_perm.astype(ml_dtypes.bfloat16)),
            "bout": np.ascontiguousarray(bout_eff).reshape(1, CPC),
        })

    kwargs = {}
    tdir = os.environ.get("KERNEL_TRACE_DIR")
    if tdir:
        kwargs = dict(trace=True, tmpdir=tdir)
    res = run_bass_kernel_spmd(
        nc, in_maps, core_ids=list(range(NCORES)), **kwargs
    )
    _CACHE["last_results"] = res

    out = np.empty((B, T, C), dtype=np.float32)
    for c in range(NCORES):
        b = c // 2
        half = slice((c % 2) * CPC, (c % 2) * CPC + CPC)
        out[b][:, half] = res.results[c]["outT"].T
    return out
